# revision 1
# baseline (speedup 1.0000x reference)
"""Trainium2 Bass kernel for nn_Attention_56470230008033.

Multi-head self-attention (B=2, N=2048, C=1024, H=16 heads, D=64),
k = v = q, full qkv projection + output projection.

Sharding over 8 NeuronCores: data parallel on batch (2) x tensor
parallel on heads (4 head-groups of 4 heads). Each core computes, for
its (batch b, head group g):
  - qkv = x @ Wqkv[:, cols(g)]         (bf16 matmul, fp32 accum)
  - per head: logits^T = K^T.T @ Q^T, P^T = exp(logits^T * 1/8)
    (no max-subtraction: logits are bounded ~|8| for this problem)
  - o_u^T / sums via [V | 1] ones-column trick, normalize
  - y_partial = o_hat @ Wproj[rows(g), :]
Host sums the 4 partials per batch and adds b_proj.
"""

import sys

for _p in ("/opt/trn_rl_repo", "/opt/pypackages"):
    if _p not in sys.path:
        sys.path.append(_p)

import numpy as np

B, N, C, H = 2, 2048, 1024, 16
D = C // H            # 64 head dim
NCORES = 8
HPC = 4               # heads per core
F = HPC * D           # 256 features per core
NT = N // 128         # 16 token tiles
CT = C // 128         # 8 contraction tiles
NCH = N // 512        # 4 free-dim chunks of 512

_CACHE = {}


def _build():
    from concourse import bacc, bass, mybir, tile, masks

    F32 = mybir.dt.float32
    BF16 = mybir.dt.bfloat16
    AF = mybir.ActivationFunctionType

    nc = bacc.Bacc(
        "TRN2",
        target_bir_lowering=False,
        debug=False,
        enable_asserts=False,
        num_devices=NCORES,
    )
    x_d = nc.dram_tensor("x", [N, C], F32, kind="ExternalInput")
    wqk_d = nc.dram_tensor("wqk", [C, 2 * F], F32, kind="ExternalInput")
    wv_d = nc.dram_tensor("wv", [C, F], F32, kind="ExternalInput")
    wp_d = nc.dram_tensor("wp", [F, C], F32, kind="ExternalInput")
    bqk_d = nc.dram_tensor("bqk", [2 * F, 1], F32, kind="ExternalInput")
    bv_d = nc.dram_tensor("bv", [1, F], F32, kind="ExternalInput")
    y_d = nc.dram_tensor("y", [N, C], F32, kind="ExternalOutput")

    with tile.TileContext(nc) as tc:
        from contextlib import ExitStack

        with ExitStack() as ctx:
            const = ctx.enter_context(tc.tile_pool(name="const", bufs=1))
            persist = ctx.enter_context(tc.tile_pool(name="persist", bufs=1))

            ident = const.tile([128, 128], BF16, name="ident", tag="ident")
            masks.make_identity(nc, ident[:])

            # persistent SBUF tensors (bf16 compute copies)
            # xTall: x^T, laid out as 8 c-tiles of [128, 2048] side by side
            xTall = persist.tile([128, CT * N], BF16, name="xTall", tag="xTall")
            wqk = [persist.tile([128, 2 * F], BF16, name=f"wqk{c}", tag=f"wqk{c}") for c in range(CT)]
            wv = [persist.tile([128, F], BF16, name=f"wv{c}", tag=f"wv{c}") for c in range(CT)]
            wp = [persist.tile([D, C], BF16, name=f"wp{h}", tag=f"wp{h}") for h in range(HPC)]
            # qkT[0..1] = Q^T tiles (256 rows), qkT[2..3] = K^T tiles
            qkT = [persist.tile([128, N], BF16, name=f"qkT{f}", tag=f"qkT{f}") for f in range(4)]
            # V with interleaved ones column per head: cols [65h .. 65h+64]
            vaug = [persist.tile([128, 65 * HPC], BF16, name=f"vaug{t}", tag=f"vaug{t}") for t in range(NT)]
            oT = [persist.tile([D, N], BF16, name=f"oT{h}", tag=f"oT{h}") for h in range(HPC)]
            bqk_sb = [const.tile([128, 1], F32, name=f"bqk{f}", tag=f"bqk{f}") for f in range(4)]
            bvb = const.tile([128, F], F32, name="bvb", tag="bvb")

            # x^T view: [128, c-tile, n]
            xT = xTall.rearrange("p (c n) -> p c n", c=CT)

            # ---- phase A: x load / transpose with V + QK projections
            # interleaved into the DMA-paced stretch (keeps the PE dense) ----
            # x rows (tp*256 + i*128 + p) -> xs[p, i*1024 + c]
            x_view = x_d.ap().rearrange("(tp i p) c -> tp p i c", tp=NT // 2, i=2)
            with tc.tile_pool(name="tpsum", bufs=2, space=bass.MemorySpace.PSUM) as tpsum, \
                 tc.tile_pool(name="vpsum", bufs=2, space=bass.MemorySpace.PSUM) as vpsum, \
                 tc.tile_pool(name="qkpsum", bufs=2, space=bass.MemorySpace.PSUM) as qkpsum, \
                 tc.tile_pool(name="xload", bufs=3) as xload, \
                 tc.tile_pool(name="xbp", bufs=2) as xbp, \
                 tc.tile_pool(name="wstage", bufs=2) as wstage:

                # preload the exp table set while the scalar engine is idle
                scr = const.tile([1, 16], F32, name="scr", tag="scr")
                nc.scalar.activation(scr[:], ident[0:1, 0:16], AF.Exp)

                # HAM warm-up: no-dep filler matmuls so the PE clock ungates
                # before the real work arrives (values are junk, never read)
                for _ in range(6):
                    wt = vpsum.tile([128, F], F32, name="wt", tag="vp")
                    for _ in range(4):
                        nc.tensor.matmul(wt[:], ident[:], wqk[0][:, 0:F],
                                         start=True, stop=True)

                def qk_proj(f, half, copy_eng="scalar"):
                    # two n-chunks per accumulator tile; lhsT (weights)
                    # reused across both chunk matmuls of each c-tile
                    qp = qkpsum.tile([128, 2, 512], F32, name="qp", tag="qp")
                    for c in range(CT):
                        for j in range(2):
                            nc.tensor.matmul(
                                qp[:, j],
                                wqk[c][:, f * 128:(f + 1) * 128],
                                xT[:, c, (2 * half + j) * 512:(2 * half + j + 1) * 512],
                                start=(c == 0), stop=(c == CT - 1))
                    for j in range(2):
                        dst = qkT[f][:, (2 * half + j) * 512:(2 * half + j + 1) * 512]
                        if copy_eng == "scalar":
                            nc.scalar.activation(dst, qp[:, j], AF.Identity,
                                                 bias=bqk_sb[f][:])
                        else:
                            nc.vector.tensor_scalar_add(dst, qp[:, j], bqk_sb[f][:])

                def v_proj(t):
                    vp = vpsum.tile([128, F], F32, name="vp", tag="vp")
                    for c in range(CT):
                        nc.tensor.matmul(
                            vp[:], xT[:, c, t * 128:(t + 1) * 128], wv[c][:],
                            start=(c == 0), stop=(c == CT - 1))
                    for h in range(HPC):
                        nc.vector.tensor_add(
                            vaug[t][:, 65 * h:65 * h + D],
                            vp[:, h * D:(h + 1) * D],
                            bvb[:, h * D:(h + 1) * D])

                for tp in range(NT // 2):
                    xs = xload.tile([128, 2048], F32, name="xs", tag="xs")
                    xsv = xs.rearrange("p (i c) -> p i c", i=2)
                    if tp < 2:
                        # finer first transfers so the PE can start sooner
                        for i in range(2):
                            nc.sync.dma_start(xsv[:, i], x_view[tp][:, i])
                    else:
                        nc.sync.dma_start(xsv, x_view[tp])
                    # weight loads on the scalar ring, front-loaded
                    if tp < 2:
                        for c in range(4 * tp, 4 * tp + 4):
                            s = wstage.tile([128, 2 * F], F32, name="wqks", tag="wqks")
                            nc.gpsimd.dma_start(s[:], wqk_d.ap()[c * 128:(c + 1) * 128, :])
                            nc.vector.tensor_copy(wqk[c][:], s[:])
                        if tp == 1:
                            for f in range(4):
                                nc.gpsimd.dma_start(bqk_sb[f][:], bqk_d.ap()[f * 128:(f + 1) * 128, :])
                    elif tp < 4:
                        for c in range(4 * (tp - 2), 4 * (tp - 2) + 4):
                            s2 = wstage.tile([128, F], F32, name="wvs", tag="wvs")
                            nc.gpsimd.dma_start(s2[:], wv_d.ap()[c * 128:(c + 1) * 128, :])
                            nc.vector.tensor_copy(wv[c][:], s2[:])
                    elif tp == 4:
                        for h in range(HPC):
                            s3 = wstage.tile([D, C], F32, name="wps", tag="wps")
                            nc.gpsimd.dma_start(s3[:], wp_d.ap()[h * D:(h + 1) * D, :])
                            nc.vector.tensor_copy(wp[h][:], s3[:])
                        bv1 = const.tile([1, F], F32, name="bv1", tag="bv1")
                        nc.gpsimd.dma_start(bv1[:], bv_d.ap()[:])
                        nc.gpsimd.partition_broadcast(bvb[:], bv1[:])
                        for t2 in range(NT):
                            for h in range(HPC):
                                nc.vector.memset(vaug[t2][:, 65 * h + 64:65 * h + 65], 1.0)
                    for i in range(2):
                        t = 2 * tp + i
                        xb = xbp.tile([128, 1024], BF16, name="xb", tag="xb")
                        nc.vector.tensor_copy(xb[:], xs[:, i * 1024:(i + 1) * 1024])
                        for cq in range(2):  # quad of 4 c-tiles
                            tq = tpsum.tile([128, 512], BF16, name="tq", tag="tq")
                            for j in range(4):
                                c = 4 * cq + j
                                nc.tensor.transpose(
                                    tq[:, j * 128:(j + 1) * 128],
                                    xb[:, c * 128:(c + 1) * 128],
                                    ident[:])
                            # scatter the quad into xT[c][:, t*128:(t+1)*128]
                            nc.scalar.copy(
                                xT[:, 4 * cq:4 * cq + 4, t * 128:(t + 1) * 128],
                                tq.rearrange("p (c n) -> p c n", c=4))
                        if tp < 4:
                            wt2 = vpsum.tile([128, F], F32, name="wt2", tag="vp")
                            for _ in range(6):
                                nc.tensor.matmul(
                                    wt2[:], ident[:], xb[:, 0:F],
                                    start=True, stop=True)
                    # first-half QK chunks once tiles 0-7 + weights landed
                    if tp >= 4:
                        qk_proj((2, 0, 3, 1)[tp - 4], 0)
                    # V projections once wv + the tiles are in
                    if tp >= 4:
                        for t in range(4 * (tp - 4), 4 * (tp - 4) + 4):
                            v_proj(t)

                for f in (2, 0, 3, 1):
                    qk_proj(f, 1, copy_eng="vector")

            # ---- phase 2: per-head attention, split in two n-halves ----
            with tc.tile_pool(name="bpsum", bufs=2, space=bass.MemorySpace.PSUM) as bpsum, \
                 tc.tile_pool(name="cpsum", bufs=2, space=bass.MemorySpace.PSUM) as cpsum, \
                 tc.tile_pool(name="ptp", bufs=10) as ptp, \
                 tc.tile_pool(name="snorm", bufs=2) as snorm:
                for h in range(HPC):
                    qt = qkT[h // 2]
                    kt = qkT[2 + h // 2]
                    rb = D * (h % 2)  # row base within the f-tile
                    for half in range(2):
                        nb = half * 1024
                        cp = cpsum.tile([65, 1024], F32, name="cp", tag="cp")
                        cwt = None
                        if h == HPC - 1 and half == 1:
                            cwt = cpsum.tile([65, 1024], F32, name="cwt", tag="cp")

                        def b_mm(mt):
                            bp = bpsum.tile([128, 1024], F32, name="bp", tag="bp")
                            for sub in range(2):
                                nc.tensor.matmul(
                                    bp[:, sub * 512:(sub + 1) * 512],
                                    kt[rb:rb + D, mt * 128:(mt + 1) * 128],
                                    qt[rb:rb + D, nb + sub * 512:nb + (sub + 1) * 512],
                                    start=True, stop=True)
                            return bp

                        # software-pipeline: B runs one step ahead of exp/C so
                        # the exp stream never waits on a fresh logits tile
                        bps = [b_mm(0), b_mm(1)]
                        for mt in range(NT):
                            pt = ptp.tile([128, 1024], BF16, name="pt", tag="pt")
                            nc.scalar.activation(pt[:], bps[mt][:], AF.Exp,
                                                 scale=float(D) ** -0.5)
                            for sub in range(2):
                                nc.tensor.matmul(
                                    cp[:, sub * 512:(sub + 1) * 512],
                                    vaug[mt][:, 65 * h:65 * h + 65],
                                    pt[:, sub * 512:(sub + 1) * 512],
                                    start=(mt == 0), stop=(mt == NT - 1))
                            if mt + 2 < NT:
                                bps.append(b_mm(mt + 2))
                            if cwt is not None and mt >= 8:
                                for sub in range(2):
                                    nc.tensor.matmul(
                                        cwt[:, sub * 512:(sub + 1) * 512],
                                        ident[:, 0:65],
                                        qkT[0][:, sub * 512:(sub + 1) * 512],
                                        start=True, stop=True)
                        s0 = snorm.tile([1, 1024], F32, name="s0", tag="s0")
                        nc.vector.tensor_copy(s0[:], cp[64:65, :])
                        sr = snorm.tile([1, 1024], F32, name="sr", tag="sr")
                        nc.vector.reciprocal_approx_fast(sr[:], s0[:])
                        sb = snorm.tile([D, 1024], F32, name="sb", tag="sb")
                        nc.gpsimd.partition_broadcast(sb[:], sr[:])
                        nc.vector.tensor_mul(oT[h][:, nb:nb + 1024], cp[0:D, :], sb[:])

            # ---- phase 3: output projection (partial, head-group rows) ----
            with tc.tile_pool(name="ypsum", bufs=3, space=bass.MemorySpace.PSUM) as ypsum, \
                 tc.tile_pool(name="ywarm", bufs=1, space=bass.MemorySpace.PSUM) as ywarm, \
                 tc.tile_pool(name="ysb", bufs=4) as ysb:
                ywt = ywarm.tile([128, 512], F32, name="ywt", tag="ywt")
                for _ in range(24):
                    nc.tensor.matmul(ywt[:], ident[:], xTall[:, 0:512],
                                     start=True, stop=True)
                for t in range(NT):
                    yp = ypsum.tile([128, 1024], F32, name="yp", tag="yp")
                    for h in range(HPC):
                        for ch in range(2):
                            nc.tensor.matmul(
                                yp[:, ch * 512:(ch + 1) * 512],
                                oT[h][:, t * 128:(t + 1) * 128],
                                wp[h][:, ch * 512:(ch + 1) * 512],
                                start=(h == 0), stop=(h == HPC - 1))
                    ys = ysb.tile([128, 1024], F32, name="ys", tag="ys")
                    if t % 2 == 0:
                        nc.vector.tensor_copy(ys[:], yp[:])
                    else:
                        nc.scalar.copy(ys[:], yp[:])
                    nc.sync.dma_start(y_d.ap()[t * 128:(t + 1) * 128, :], ys[:])

    nc.compile()
    return nc


def _get_nc():
    if "nc" not in _CACHE:
        _CACHE["nc"] = _build()
    return _CACHE["nc"]


def _in_maps(q, W_qkv, b_qkv, W_proj):
    maps = []
    for core in range(NCORES):
        b, g = divmod(core, HPC)
        cols = slice(g * F, (g + 1) * F)
        maps.append({
            "x": q[b],
            "wqk": np.ascontiguousarray(
                np.concatenate([W_qkv[:, cols], W_qkv[:, C:][:, cols]], axis=1)),
            "wv": np.ascontiguousarray(W_qkv[:, 2 * C:][:, cols]),
            "wp": np.ascontiguousarray(W_proj[cols, :]),
            "bqk": np.ascontiguousarray(
                np.concatenate([b_qkv[cols], b_qkv[C:][cols]]).reshape(2 * F, 1)),
            "bv": np.ascontiguousarray(b_qkv[2 * C:][cols].reshape(1, F)),
        })
    return maps


def kernel(q, W_qkv, b_qkv, W_proj, b_proj):
    from concourse.bass_utils import run_bass_kernel_spmd

    q = np.ascontiguousarray(np.asarray(q, dtype=np.float32))
    W_qkv = np.ascontiguousarray(np.asarray(W_qkv, dtype=np.float32))
    b_qkv = np.ascontiguousarray(np.asarray(b_qkv, dtype=np.float32))
    W_proj = np.ascontiguousarray(np.asarray(W_proj, dtype=np.float32))
    b_proj = np.ascontiguousarray(np.asarray(b_proj, dtype=np.float32))

    nc = _get_nc()
    res = run_bass_kernel_spmd(nc, _in_maps(q, W_qkv, b_qkv, W_proj),
                               core_ids=list(range(NCORES)))

    out = np.zeros((B, N, C), dtype=np.float32)
    for core in range(NCORES):
        out[core // HPC] += res.results[core]["y"]
    out += b_proj
    return out



# revision 8
# speedup vs baseline: 1.1711x; 1.1711x over previous
"""Trainium2 Bass kernel for nn_Attention_56470230008033.

Multi-head self-attention (B=2, N=2048, C=1024, H=16 heads, D=64),
k = v = q, full qkv projection + output projection.

Sharding over 8 NeuronCores: data parallel on batch (2) x tensor
parallel on heads (4 head-groups of 4 heads).

Fused streaming design (v2): the scalar engine's exp stream over the
4x2048x2048 attention matrix (~110us of ACT work) is the critical
path, so everything else is scheduled around keeping it dense:
  - x and weights are passed from host in bf16; x is DMA'd with the
    XBAR transpose directly DRAM -> SBUF (no staging, no PE transpose)
  - logits^T per head pair via row-tiled concurrent K=64 matmuls
    (heads at PE rows 0-63 / 64-127, separate PSUM banks)
  - softmax denominators via the ones-column-in-V trick (65-col PV)
  - output projection with K=128 (two heads packed per contraction)
  - per-chunk (512 query rows) pipeline: B -> exp -> PV, with the
    QKV projections for later pairs interleaved into PE slack, and
    y projection + DMA-out streamed per chunk of the second pair
"""

import os
import sys

for _p in ("/opt/trn_rl_repo", "/opt/pypackages"):
    if _p not in sys.path:
        sys.path.append(_p)

import numpy as np

_DEBUG = os.environ.get("KDEBUG") == "1"

B, N, C, H = 2, 2048, 1024, 16
D = C // H            # 64 head dim
NCORES = 8
HPC = 4               # heads per core
F = HPC * D           # 256 features per core
NT = N // 128         # 16 token tiles
CT = C // 128         # 8 contraction tiles
NCH = N // 512        # 4 chunks of 512

_CACHE = {}


def _build():
    from concourse import bacc, bass, mybir, tile, masks

    F32 = mybir.dt.float32
    BF16 = mybir.dt.bfloat16
    AF = mybir.ActivationFunctionType

    nc = bacc.Bacc(
        "TRN2",
        target_bir_lowering=False,
        debug=False,
        enable_asserts=False,
        num_devices=NCORES,
    )
    x_d = nc.dram_tensor("x", [N, C], BF16, kind="ExternalInput")
    # cols = [Q01 | K01 | Q23 | K23], 128 each (local head pairs)
    wqk_d = nc.dram_tensor("wqk", [C, 4 * 128], BF16, kind="ExternalInput")
    wv_d = nc.dram_tensor("wv", [C, F], BF16, kind="ExternalInput")
    wp_d = nc.dram_tensor("wp", [F, C], BF16, kind="ExternalInput")
    bqk_d = nc.dram_tensor("bqk", [4 * 128, 1], F32, kind="ExternalInput")
    bv_d = nc.dram_tensor("bv", [1, F], F32, kind="ExternalInput")
    y_d = nc.dram_tensor("y", [N, C], F32, kind="ExternalOutput")
    if _DEBUG:
        qk_dump = nc.dram_tensor("qk_dump", [4 * 128, N], BF16, kind="ExternalOutput")
        va_dump = nc.dram_tensor("va_dump", [NT * 128, HPC * (D + 1)], BF16,
                                 kind="ExternalOutput")
        ot_dump = nc.dram_tensor("ot_dump", [2 * 128, N], BF16, kind="ExternalOutput")

    with tile.TileContext(nc) as tc:
        from contextlib import ExitStack

        with ExitStack() as ctx:
            const = ctx.enter_context(tc.tile_pool(name="const", bufs=1))
            persist = ctx.enter_context(tc.tile_pool(name="persist", bufs=1))
            ptp = ctx.enter_context(tc.tile_pool(name="ptp", bufs=3))
            ysb = ctx.enter_context(tc.tile_pool(name="ysb", bufs=2))
            snorm = ctx.enter_context(tc.tile_pool(name="snorm", bufs=2))

            ident = const.tile([128, 128], BF16, name="ident", tag="ident")
            masks.make_identity(nc, ident[:])
            junk = const.tile([128, 512], BF16, name="junk", tag="junk")
            nc.vector.memset(junk[:], 0.0)

            # persistent SBUF tensors (all bf16 from host)
            xTall = persist.tile([128, CT * N], BF16, name="xTall", tag="xTall")
            xT3 = xTall.rearrange("p (c n) -> p c n", c=CT)
            wqk = [persist.tile([128, 4 * 128], BF16, name=f"wqk{c}", tag=f"wqk{c}")
                   for c in range(CT)]
            wv = [persist.tile([128, F], BF16, name=f"wv{c}", tag=f"wv{c}")
                  for c in range(CT)]
            wpp = [persist.tile([128, C], BF16, name=f"wpp{hp}", tag=f"wpp{hp}")
                   for hp in range(2)]
            # qkT[0]=Q01 qkT[1]=K01 qkT[2]=Q23 qkT[3]=K23; per pair the
            # even head sits at rows 0-63, odd head at rows 64-127
            qkT = [persist.tile([128, N], BF16, name=f"qkT{f}", tag=f"qkT{f}")
                   for f in range(4)]
            # V with ones column per head: [128, h, 65]
            vaug = [persist.tile([128, HPC, D + 1], BF16, name=f"vaug{t}", tag=f"vaug{t}")
                    for t in range(NT)]
            oTp = [persist.tile([128, N], BF16, name=f"oTp{hp}", tag=f"oTp{hp}")
                   for hp in range(2)]
            bqk_sb = [const.tile([128, 1], F32, name=f"bqk{f}", tag=f"bqk{f}")
                      for f in range(4)]
            bvb = const.tile([128, F], F32, name="bvb", tag="bvb")
            bvb3 = bvb.rearrange("p (h d) -> p h d", h=HPC)

            # ---- front-loaded DMA issue ----
            # weights on the gpsimd (SWDGE) ring, in need-order
            for c in range(CT):
                nc.gpsimd.dma_start(wqk[c][:, 0:256],
                                    wqk_d.ap()[c * 128:(c + 1) * 128, 0:256])
            # x: whole-tensor XBAR transpose in 4 chunks on the sync ring
            for g in range(4):
                nc.sync.dma_start(xT3[:, :, g * 512:(g + 1) * 512],
                                  x_d.ap()[g * 512:(g + 1) * 512, :],
                                  transpose=True)
            for c in range(CT):
                nc.gpsimd.dma_start(wv[c][:], wv_d.ap()[c * 128:(c + 1) * 128, :])
            for c in range(CT):
                nc.gpsimd.dma_start(wqk[c][:, 256:512],
                                    wqk_d.ap()[c * 128:(c + 1) * 128, 256:512])
            for hp in range(2):
                nc.gpsimd.dma_start(wpp[hp][:], wp_d.ap()[hp * 128:(hp + 1) * 128, :])
            for f in range(4):
                nc.gpsimd.dma_start(bqk_sb[f][:], bqk_d.ap()[f * 128:(f + 1) * 128, :])
            bv1 = const.tile([1, F], F32, name="bv1", tag="bv1")
            nc.gpsimd.dma_start(bv1[:], bv_d.ap()[:])
            nc.gpsimd.partition_broadcast(bvb[:], bv1[:])

            # exp table preload on the scalar engine (one-time ~2.7us)
            scr = const.tile([1, 16], F32, name="scr", tag="scr")
            nc.scalar.activation(scr[:], ident[0:1, 0:16], AF.Exp)

            # ones columns of vaug (never overwritten afterwards)
            for t in range(NT):
                nc.vector.memset(vaug[t][:, :, D:D + 1], 1.0)

            with tc.tile_pool(name="bpp", bufs=2, space=bass.MemorySpace.PSUM) as bpp, \
                 tc.tile_pool(name="cpp", bufs=1, space=bass.MemorySpace.PSUM) as cpp:

                def qk_proj(f, j):
                    # project qkT[f] n-cols j*512:(j+1)*512
                    qp = qpp.tile([128, 512], F32, name="qp", tag="qp")
                    for c in range(CT):
                        nc.tensor.matmul(
                            qp[:], wqk[c][:, f * 128:(f + 1) * 128],
                            xT3[:, c, j * 512:(j + 1) * 512],
                            start=(c == 0), stop=(c == CT - 1))
                    nc.vector.tensor_scalar_add(
                        qkT[f][:, j * 512:(j + 1) * 512], qp[:], bqk_sb[f][:])

                def v_proj(t):
                    vp = vpp.tile([128, F], F32, name="vp", tag="vp")
                    for c in range(CT):
                        nc.tensor.matmul(
                            vp[:], xT3[:, c, t * 128:(t + 1) * 128], wv[c][:],
                            start=(c == 0), stop=(c == CT - 1))
                    nc.vector.tensor_add(
                        vaug[t][:, :, 0:D],
                        vp.rearrange("p (h d) -> p h d", h=HPC), bvb3)

                def y_sub(c, s):
                    t = c * 4 + s
                    yp = ypp.tile([128, 2, 512], F32, name="yp", tag="yp")
                    for hp in range(2):
                        for half in range(2):
                            nc.tensor.matmul(
                                yp[:, half],
                                oTp[hp][:, t * 128:(t + 1) * 128],
                                wpp[hp][:, half * 512:(half + 1) * 512],
                                start=(hp == 0), stop=(hp == 1))
                    ys = ysb.tile([128, C], F32, name="ys", tag="ys")
                    nc.vector.tensor_copy(ys[:], yp.rearrange("p a b -> p (a b)"))
                    nc.sync.dma_start(y_d.ap()[t * 128:(t + 1) * 128, :], ys[:])

                def run_chunk(p, c, hooks):
                    qt, kt = qkT[2 * p], qkT[2 * p + 1]
                    nb = c * 512
                    cp = cpp.tile([D + 1, 2, 512], F32, name="cp", tag="cp")

                    def bmm(mt):
                        bp = bpp.tile([128, 2, 512], F32, name="bp", tag="bp")
                        nc.tensor.matmul(
                            bp[:, 0], kt[0:D, mt * 128:(mt + 1) * 128],
                            qt[0:D, nb:nb + 512], start=True, stop=True)
                        nc.tensor.matmul(
                            bp[:, 1], kt[D:2 * D, mt * 128:(mt + 1) * 128],
                            qt[D:2 * D, nb:nb + 512], start=True, stop=True)
                        return bp

                    bps = {0: bmm(0), 1: bmm(1)}
                    for mt in range(NT):
                        pt = ptp.tile([128, 2, 512], BF16, name="pt", tag="pt")
                        nc.scalar.activation(
                            pt.rearrange("p a b -> p (a b)"),
                            bps.pop(mt).rearrange("p a b -> p (a b)"),
                            AF.Exp, scale=float(D) ** -0.5)
                        nc.tensor.matmul(cp[:, 0], vaug[mt][:, 2 * p, :], pt[:, 0],
                                         start=(mt == 0), stop=(mt == NT - 1))
                        nc.tensor.matmul(cp[:, 1], vaug[mt][:, 2 * p + 1, :], pt[:, 1],
                                         start=(mt == 0), stop=(mt == NT - 1))
                        if mt + 2 < NT:
                            bps[mt + 2] = bmm(mt + 2)
                        for hook in hooks.get(mt, ()):
                            hook()
                    # normalize: oTp rows = cp[0:D] * (1/cp[D]) per head.
                    # DVE lanes are partition-locked, so the odd head is
                    # normalized into a partition-0 scratch and moved to
                    # partitions 64-127 with a SBUF->SBUF DMA.
                    for ho in range(2):
                        # copy the den row to partition 0 first: DVE
                        # tensor_copy honors cross-partition moves,
                        # reciprocal does not (reads partition 0 regardless)
                        s0 = snorm.tile([1, 512], F32, name=f"s0{ho}", tag=f"s0{ho}")
                        nc.vector.tensor_copy(s0[:], cp[D:D + 1, ho])
                        sr = snorm.tile([1, 512], F32, name=f"sr{ho}", tag=f"sr{ho}")
                        nc.vector.reciprocal_approx_fast(sr[:], s0[:])
                        sb = snorm.tile([D, 512], F32, name=f"sb{ho}", tag=f"sb{ho}")
                        nc.gpsimd.partition_broadcast(sb[:], sr[:])
                        if ho == 0:
                            nc.vector.tensor_mul(
                                oTp[p][0:D, nb:nb + 512], cp[0:D, 0], sb[:])
                        else:
                            om = snorm.tile([D, 512], BF16, name="om", tag="om")
                            nc.vector.tensor_mul(om[:], cp[0:D, 1], sb[:])
                            nc.gpsimd.dma_start(
                                oTp[p][D:2 * D, nb:nb + 512], om[:])

                # ---- pair 0: projections + chunk stream ----
                with tc.tile_pool(name="qpp", bufs=1, space=bass.MemorySpace.PSUM) as qpp, \
                     tc.tile_pool(name="vpp", bufs=1, space=bass.MemorySpace.PSUM) as vpp:
                    # HAM warm-up fillers (junk, never read)
                    for _ in range(8):
                        wt = qpp.tile([128, 512], F32, name="wt", tag="qp")
                        nc.tensor.matmul(wt[:], ident[:], junk[:],
                                         start=True, stop=True)
                    qk_proj(0, 0)      # Q01 chunk 0
                    qk_proj(1, 0)      # K01 m-chunk 0
                    v_proj(0)
                    v_proj(1)

                    hooks0 = {mt: [(lambda t=mt + 2: v_proj(t))]
                              for mt in range(NT - 2)}
                    for mk, mt in ((1, 1), (2, 5), (3, 9)):
                        hooks0[mt].append(lambda mk=mk: qk_proj(1, mk))
                    hooks0[12].append(lambda: qk_proj(0, 1))
                    run_chunk(0, 0, hooks0)
                    run_chunk(0, 1, {2: [lambda: qk_proj(3, 0)],
                                     6: [lambda: qk_proj(3, 1)],
                                     10: [lambda: qk_proj(0, 2)]})
                    run_chunk(0, 2, {2: [lambda: qk_proj(3, 2)],
                                     6: [lambda: qk_proj(3, 3)],
                                     10: [lambda: qk_proj(0, 3)]})
                    run_chunk(0, 3, {2: [lambda: qk_proj(2, 0)],
                                     5: [lambda: qk_proj(2, 1)],
                                     8: [lambda: qk_proj(2, 2)],
                                     11: [lambda: qk_proj(2, 3)]})

                # ---- pair 1: chunk stream + streamed y projection ----
                with tc.tile_pool(name="ypp", bufs=1, space=bass.MemorySpace.PSUM) as ypp:
                    run_chunk(1, 0, {})
                    run_chunk(1, 1, {mt: [(lambda s=s: y_sub(0, s))]
                                     for s, mt in enumerate((2, 5, 8, 11))})
                    run_chunk(1, 2, {mt: [(lambda s=s: y_sub(1, s))]
                                     for s, mt in enumerate((2, 5, 8, 11))})
                    run_chunk(1, 3, {mt: [(lambda s=s: y_sub(2, s))]
                                     for s, mt in enumerate((2, 5, 8, 11))})
                    for s in range(4):
                        y_sub(3, s)

                if _DEBUG:
                    for f in range(4):
                        nc.sync.dma_start(qk_dump.ap()[f * 128:(f + 1) * 128, :],
                                          qkT[f][:])
                    for t in range(NT):
                        nc.sync.dma_start(va_dump.ap()[t * 128:(t + 1) * 128, :],
                                          vaug[t].rearrange("p h d -> p (h d)"))
                    for hp in range(2):
                        nc.sync.dma_start(ot_dump.ap()[hp * 128:(hp + 1) * 128, :],
                                          oTp[hp][:])

    nc.compile()
    return nc


def _get_nc():
    if "nc" not in _CACHE:
        _CACHE["nc"] = _build()
    return _CACHE["nc"]


def _in_maps(q, W_qkv, b_qkv, W_proj):
    import ml_dtypes

    bf16 = ml_dtypes.bfloat16
    maps = []
    Wq, Wk, Wv = W_qkv[:, :C], W_qkv[:, C:2 * C], W_qkv[:, 2 * C:]
    bq, bk, bv = b_qkv[:C], b_qkv[C:2 * C], b_qkv[2 * C:]
    for core in range(NCORES):
        b, g = divmod(core, HPC)
        cols = slice(g * F, (g + 1) * F)
        wqg, wkg = Wq[:, cols], Wk[:, cols]
        maps.append({
            "x": np.ascontiguousarray(q[b].astype(bf16)),
            "wqk": np.ascontiguousarray(np.concatenate(
                [wqg[:, 0:128], wkg[:, 0:128], wqg[:, 128:256], wkg[:, 128:256]],
                axis=1).astype(bf16)),
            "wv": np.ascontiguousarray(Wv[:, cols].astype(bf16)),
            "wp": np.ascontiguousarray(W_proj[cols, :].astype(bf16)),
            "bqk": np.ascontiguousarray(np.concatenate(
                [bq[cols][0:128], bk[cols][0:128],
                 bq[cols][128:256], bk[cols][128:256]]).reshape(4 * 128, 1)),
            "bv": np.ascontiguousarray(bv[cols].reshape(1, F)),
        })
    return maps


def kernel(q, W_qkv, b_qkv, W_proj, b_proj):
    from concourse.bass_utils import run_bass_kernel_spmd

    q = np.ascontiguousarray(np.asarray(q, dtype=np.float32))
    W_qkv = np.ascontiguousarray(np.asarray(W_qkv, dtype=np.float32))
    b_qkv = np.ascontiguousarray(np.asarray(b_qkv, dtype=np.float32))
    W_proj = np.ascontiguousarray(np.asarray(W_proj, dtype=np.float32))
    b_proj = np.ascontiguousarray(np.asarray(b_proj, dtype=np.float32))

    nc = _get_nc()
    res = run_bass_kernel_spmd(nc, _in_maps(q, W_qkv, b_qkv, W_proj),
                               core_ids=list(range(NCORES)))

    out = np.zeros((B, N, C), dtype=np.float32)
    for core in range(NCORES):
        out[core // HPC] += res.results[core]["y"]
    out += b_proj
    return out


# revision 13
# speedup vs baseline: 1.3398x; 1.1441x over previous
"""Trainium2 Bass kernel for nn_Attention_56470230008033.

Multi-head self-attention (B=2, N=2048, C=1024, H=16 heads, D=64),
k = v = q, full qkv projection + output projection.

Sharding over 8 NeuronCores: data parallel on batch (2) x tensor
parallel on heads (4 head-groups of 4 heads).

Fused streaming design (v2): the scalar engine's exp stream over the
4x2048x2048 attention matrix (~110us of ACT work) is the critical
path, so everything else is scheduled around keeping it dense:
  - x and weights are passed from host in bf16; x is DMA'd with the
    XBAR transpose directly DRAM -> SBUF (no staging, no PE transpose)
  - logits^T per head pair via row-tiled concurrent K=64 matmuls
    (heads at PE rows 0-63 / 64-127, separate PSUM banks)
  - softmax denominators via the ones-column-in-V trick (65-col PV)
  - output projection with K=128 (two heads packed per contraction)
  - per-chunk (512 query rows) pipeline: B -> exp -> PV, with the
    QKV projections for later pairs interleaved into PE slack, and
    y projection + DMA-out streamed per chunk of the second pair
"""

import os
import sys

for _p in ("/opt/trn_rl_repo", "/opt/pypackages"):
    if _p not in sys.path:
        sys.path.append(_p)

import numpy as np

_DEBUG = os.environ.get("KDEBUG") == "1"

B, N, C, H = 2, 2048, 1024, 16
D = C // H            # 64 head dim
NCORES = 8
HPC = 4               # heads per core
F = HPC * D           # 256 features per core
NT = N // 128         # 16 token tiles
CT = C // 128         # 8 contraction tiles
NCH = N // 512        # 4 chunks of 512

_CACHE = {}


def _build():
    from concourse import bacc, bass, mybir, tile, masks

    F32 = mybir.dt.float32
    BF16 = mybir.dt.bfloat16
    AF = mybir.ActivationFunctionType

    nc = bacc.Bacc(
        "TRN2",
        target_bir_lowering=False,
        debug=False,
        enable_asserts=False,
        num_devices=NCORES,
    )
    x_d = nc.dram_tensor("x", [N, C], BF16, kind="ExternalInput")
    # cols = [Q01 | K01 | Q23 | K23], 128 each (local head pairs)
    wqk_d = nc.dram_tensor("wqk", [C, 4 * 128], BF16, kind="ExternalInput")
    wv_d = nc.dram_tensor("wv", [C, F], BF16, kind="ExternalInput")
    wp_d = nc.dram_tensor("wp", [F, C], BF16, kind="ExternalInput")
    bqk_d = nc.dram_tensor("bqk", [4 * 128, 1], F32, kind="ExternalInput")
    bv_d = nc.dram_tensor("bv", [1, F], F32, kind="ExternalInput")
    y_d = nc.dram_tensor("y", [N, C], F32, kind="ExternalOutput")
    if _DEBUG:
        qk_dump = nc.dram_tensor("qk_dump", [4 * 128, N], BF16, kind="ExternalOutput")
        va_dump = nc.dram_tensor("va_dump", [NT * 128, HPC * (D + 1)], BF16,
                                 kind="ExternalOutput")
        ot_dump = nc.dram_tensor("ot_dump", [2 * 128, N], BF16, kind="ExternalOutput")

    with tile.TileContext(nc) as tc:
        from contextlib import ExitStack

        with ExitStack() as ctx:
            const = ctx.enter_context(tc.tile_pool(name="const", bufs=1))
            persist = ctx.enter_context(tc.tile_pool(name="persist", bufs=1))
            ptp = ctx.enter_context(tc.tile_pool(name="ptp", bufs=6))
            ysb = ctx.enter_context(tc.tile_pool(name="ysb", bufs=2))
            snorm = ctx.enter_context(tc.tile_pool(name="snorm", bufs=2))

            ident = const.tile([128, 128], BF16, name="ident", tag="ident")
            masks.make_identity(nc, ident[:])
            junk = const.tile([128, 512], BF16, name="junk", tag="junk")
            nc.vector.memset(junk[:], 0.0)

            # persistent SBUF tensors (all bf16 from host)
            xTall = persist.tile([128, CT * N], BF16, name="xTall", tag="xTall")
            xT3 = xTall.rearrange("p (c n) -> p c n", c=CT)
            wqk = [persist.tile([128, 4 * 128], BF16, name=f"wqk{c}", tag=f"wqk{c}")
                   for c in range(CT)]
            wv = [persist.tile([128, F], BF16, name=f"wv{c}", tag=f"wv{c}")
                  for c in range(CT)]
            wpp = [persist.tile([128, C], BF16, name=f"wpp{hp}", tag=f"wpp{hp}")
                   for hp in range(2)]
            # qkT[0]=Q01 qkT[1]=K01 qkT[2]=Q23 qkT[3]=K23; per pair the
            # even head sits at rows 0-63, odd head at rows 64-127
            qkT = [persist.tile([128, N], BF16, name=f"qkT{f}", tag=f"qkT{f}")
                   for f in range(4)]
            # V with ones column per head: [128, h, 65]
            vaug = [persist.tile([128, HPC, D + 1], BF16, name=f"vaug{t}", tag=f"vaug{t}")
                    for t in range(NT)]
            oTp = [persist.tile([128, N], BF16, name=f"oTp{hp}", tag=f"oTp{hp}")
                   for hp in range(2)]
            bqk_sb = [const.tile([128, 1], F32, name=f"bqk{f}", tag=f"bqk{f}")
                      for f in range(4)]
            bvb = const.tile([128, F], F32, name="bvb", tag="bvb")
            bvb3 = bvb.rearrange("p (h d) -> p h d", h=HPC)

            # ---- front-loaded DMA issue, all on the sync (HWDGE) ring.
            # DMA_TRANSPOSE serializes against other DMA traffic, so the
            # weight loads are interleaved between the 4 x-transposes in
            # consumption order rather than on a second ring.
            def xpose(g):
                nc.sync.dma_start(xT3[:, :, g * 512:(g + 1) * 512],
                                  x_d.ap()[g * 512:(g + 1) * 512, :],
                                  transpose=True)

            for c in range(CT):   # Q01|K01 columns first
                nc.sync.dma_start(wqk[c][:, 0:256],
                                  wqk_d.ap()[c * 128:(c + 1) * 128, 0:256])
            for f in range(4):
                nc.sync.dma_start(bqk_sb[f][:], bqk_d.ap()[f * 128:(f + 1) * 128, :])
            xpose(0)
            for c in range(CT):
                nc.sync.dma_start(wv[c][:], wv_d.ap()[c * 128:(c + 1) * 128, :])
            bv1 = const.tile([1, F], F32, name="bv1", tag="bv1")
            nc.sync.dma_start(bv1[:], bv_d.ap()[:])
            nc.gpsimd.partition_broadcast(bvb[:], bv1[:])
            xpose(1)
            for c in range(CT):
                nc.sync.dma_start(wqk[c][:, 256:512],
                                  wqk_d.ap()[c * 128:(c + 1) * 128, 256:512])
            xpose(2)
            for hp in range(2):
                nc.sync.dma_start(wpp[hp][:], wp_d.ap()[hp * 128:(hp + 1) * 128, :])
            xpose(3)

            # exp table preload on the scalar engine (one-time ~2.7us)
            scr = const.tile([1, 16], F32, name="scr", tag="scr")
            nc.scalar.activation(scr[:], ident[0:1, 0:16], AF.Exp)

            # ones columns of vaug (never overwritten afterwards)
            for t in range(NT):
                nc.vector.memset(vaug[t][:, :, D:D + 1], 1.0)

            with tc.tile_pool(name="bpp", bufs=2, space=bass.MemorySpace.PSUM) as bpp, \
                 tc.tile_pool(name="cpp", bufs=1, space=bass.MemorySpace.PSUM) as cpp:

                def qk_proj(f, j):
                    # project qkT[f] n-cols j*512:(j+1)*512
                    qp = qpp.tile([128, 512], F32, name="qp", tag="qp")
                    for c in range(CT):
                        nc.tensor.matmul(
                            qp[:], wqk[c][:, f * 128:(f + 1) * 128],
                            xT3[:, c, j * 512:(j + 1) * 512],
                            start=(c == 0), stop=(c == CT - 1))
                    nc.vector.tensor_scalar_add(
                        qkT[f][:, j * 512:(j + 1) * 512], qp[:], bqk_sb[f][:])

                def v_proj(t):
                    vp = vpp.tile([128, F], F32, name="vp", tag="vp")
                    for c in range(CT):
                        nc.tensor.matmul(
                            vp[:], xT3[:, c, t * 128:(t + 1) * 128], wv[c][:],
                            start=(c == 0), stop=(c == CT - 1))
                    nc.vector.tensor_add(
                        vaug[t][:, :, 0:D],
                        vp.rearrange("p (h d) -> p h d", h=HPC), bvb3)

                def y_sub(c, s):
                    # per-512-col halves so the psum copy of one half
                    # overlaps the projection of the next
                    t = c * 4 + s
                    ys = ysb.tile([128, C], F32, name="ys", tag="ys")
                    for half in range(2):
                        yp = ypp.tile([128, 512], F32, name="yp", tag="yp")
                        for hp in range(2):
                            nc.tensor.matmul(
                                yp[:],
                                oTp[hp][:, t * 128:(t + 1) * 128],
                                wpp[hp][:, half * 512:(half + 1) * 512],
                                start=(hp == 0), stop=(hp == 1))
                        nc.vector.tensor_copy(ys[:, half * 512:(half + 1) * 512], yp[:])
                    nc.sync.dma_start(y_d.ap()[t * 128:(t + 1) * 128, :], ys[:])

                def run_chunk(p, c, hooks, pre=()):
                    qt, kt = qkT[2 * p], qkT[2 * p + 1]
                    nb = c * 512
                    cp = cpp.tile([D + 1, 2, 512], F32, name="cp", tag="cp")

                    def bmm(mt):
                        bp = bpp.tile([128, 2, 512], F32, name="bp", tag="bp")
                        nc.tensor.matmul(
                            bp[:, 0], kt[0:D, mt * 128:(mt + 1) * 128],
                            qt[0:D, nb:nb + 512], start=True, stop=True)
                        nc.tensor.matmul(
                            bp[:, 1], kt[D:2 * D, mt * 128:(mt + 1) * 128],
                            qt[D:2 * D, nb:nb + 512], start=True, stop=True)
                        return bp

                    bps = {0: bmm(0), 1: bmm(1)}
                    for hook in pre:
                        hook()
                    for mt in range(NT):
                        pt = ptp.tile([128, 2, 512], BF16, name="pt", tag="pt")
                        nc.scalar.activation(
                            pt.rearrange("p a b -> p (a b)"),
                            bps.pop(mt).rearrange("p a b -> p (a b)"),
                            AF.Exp, scale=float(D) ** -0.5)
                        nc.tensor.matmul(cp[:, 0], vaug[mt][:, 2 * p, :], pt[:, 0],
                                         start=(mt == 0), stop=(mt == NT - 1))
                        nc.tensor.matmul(cp[:, 1], vaug[mt][:, 2 * p + 1, :], pt[:, 1],
                                         start=(mt == 0), stop=(mt == NT - 1))
                        if mt + 2 < NT:
                            bps[mt + 2] = bmm(mt + 2)
                        for hook in hooks.get(mt, ()):
                            hook()
                    # normalize: oTp rows = cp[0:D] * (1/cp[D]) per head.
                    # DVE lanes are partition-locked, so the odd head is
                    # normalized into a partition-0 scratch and moved to
                    # partitions 64-127 with a SBUF->SBUF DMA.
                    for ho in range(2):
                        # copy the den row to partition 0 first: DVE
                        # tensor_copy honors cross-partition moves,
                        # reciprocal does not (reads partition 0 regardless)
                        s0 = snorm.tile([1, 512], F32, name=f"s0{ho}", tag=f"s0{ho}")
                        nc.vector.tensor_copy(s0[:], cp[D:D + 1, ho])
                        sr = snorm.tile([1, 512], F32, name=f"sr{ho}", tag=f"sr{ho}")
                        nc.vector.reciprocal_approx_fast(sr[:], s0[:])
                        sb = snorm.tile([D, 512], F32, name=f"sb{ho}", tag=f"sb{ho}")
                        nc.gpsimd.partition_broadcast(sb[:], sr[:])
                        if ho == 0:
                            nc.vector.tensor_mul(
                                oTp[p][0:D, nb:nb + 512], cp[0:D, 0], sb[:])
                        else:
                            om = snorm.tile([D, 512], BF16, name="om", tag="om")
                            nc.vector.tensor_mul(om[:], cp[0:D, 1], sb[:])
                            nc.gpsimd.dma_start(
                                oTp[p][D:2 * D, nb:nb + 512], om[:])

                # ---- pair 0: projections + chunk stream ----
                with tc.tile_pool(name="qpp", bufs=1, space=bass.MemorySpace.PSUM) as qpp, \
                     tc.tile_pool(name="vpp", bufs=1, space=bass.MemorySpace.PSUM) as vpp:
                    # HAM warm-up fillers (junk, never read)
                    for _ in range(8):
                        wt = qpp.tile([128, 512], F32, name="wt", tag="qp")
                        nc.tensor.matmul(wt[:], ident[:], junk[:],
                                         start=True, stop=True)
                    qk_proj(0, 0)      # Q01 chunk 0
                    qk_proj(1, 0)      # K01 m-chunk 0

                    hooks0 = {mt: [(lambda t=mt + 2: v_proj(t))]
                              for mt in range(NT - 2)}
                    for mk, mt in ((1, 1), (2, 5), (3, 9)):
                        hooks0[mt].append(lambda mk=mk: qk_proj(1, mk))
                    hooks0[12].append(lambda: qk_proj(0, 1))
                    run_chunk(0, 0, hooks0,
                              pre=(lambda: v_proj(0), lambda: v_proj(1)))
                    run_chunk(0, 1, {2: [lambda: qk_proj(3, 0)],
                                     6: [lambda: qk_proj(3, 1)],
                                     10: [lambda: qk_proj(0, 2)]})
                    run_chunk(0, 2, {2: [lambda: qk_proj(3, 2)],
                                     6: [lambda: qk_proj(3, 3)],
                                     10: [lambda: qk_proj(0, 3)]})
                    run_chunk(0, 3, {2: [lambda: qk_proj(2, 0)],
                                     5: [lambda: qk_proj(2, 1)],
                                     8: [lambda: qk_proj(2, 2)],
                                     11: [lambda: qk_proj(2, 3)]})

                # ---- pair 1: chunk stream + streamed y projection ----
                with tc.tile_pool(name="ypp", bufs=1, space=bass.MemorySpace.PSUM) as ypp:
                    run_chunk(1, 0, {})
                    run_chunk(1, 1, {mt: [(lambda s=s: y_sub(0, s))]
                                     for s, mt in enumerate((2, 5, 8, 11))})
                    run_chunk(1, 2, {mt: [(lambda s=s: y_sub(1, s))]
                                     for s, mt in enumerate((2, 5, 8, 11))})
                    run_chunk(1, 3, {mt: [(lambda s=s: y_sub(2, s))]
                                     for s, mt in enumerate((2, 5, 8, 11))})
                    for s in range(4):
                        y_sub(3, s)

                if _DEBUG:
                    for f in range(4):
                        nc.sync.dma_start(qk_dump.ap()[f * 128:(f + 1) * 128, :],
                                          qkT[f][:])
                    for t in range(NT):
                        nc.sync.dma_start(va_dump.ap()[t * 128:(t + 1) * 128, :],
                                          vaug[t].rearrange("p h d -> p (h d)"))
                    for hp in range(2):
                        nc.sync.dma_start(ot_dump.ap()[hp * 128:(hp + 1) * 128, :],
                                          oTp[hp][:])

    nc.compile()
    return nc


def _get_nc():
    if "nc" not in _CACHE:
        _CACHE["nc"] = _build()
    return _CACHE["nc"]


def _in_maps(q, W_qkv, b_qkv, W_proj):
    import ml_dtypes

    bf16 = ml_dtypes.bfloat16
    maps = []
    Wq, Wk, Wv = W_qkv[:, :C], W_qkv[:, C:2 * C], W_qkv[:, 2 * C:]
    bq, bk, bv = b_qkv[:C], b_qkv[C:2 * C], b_qkv[2 * C:]
    for core in range(NCORES):
        b, g = divmod(core, HPC)
        cols = slice(g * F, (g + 1) * F)
        wqg, wkg = Wq[:, cols], Wk[:, cols]
        maps.append({
            "x": np.ascontiguousarray(q[b].astype(bf16)),
            "wqk": np.ascontiguousarray(np.concatenate(
                [wqg[:, 0:128], wkg[:, 0:128], wqg[:, 128:256], wkg[:, 128:256]],
                axis=1).astype(bf16)),
            "wv": np.ascontiguousarray(Wv[:, cols].astype(bf16)),
            "wp": np.ascontiguousarray(W_proj[cols, :].astype(bf16)),
            "bqk": np.ascontiguousarray(np.concatenate(
                [bq[cols][0:128], bk[cols][0:128],
                 bq[cols][128:256], bk[cols][128:256]]).reshape(4 * 128, 1)),
            "bv": np.ascontiguousarray(bv[cols].reshape(1, F)),
        })
    return maps


def kernel(q, W_qkv, b_qkv, W_proj, b_proj):
    from concourse.bass_utils import run_bass_kernel_spmd

    q = np.ascontiguousarray(np.asarray(q, dtype=np.float32))
    W_qkv = np.ascontiguousarray(np.asarray(W_qkv, dtype=np.float32))
    b_qkv = np.ascontiguousarray(np.asarray(b_qkv, dtype=np.float32))
    W_proj = np.ascontiguousarray(np.asarray(W_proj, dtype=np.float32))
    b_proj = np.ascontiguousarray(np.asarray(b_proj, dtype=np.float32))

    nc = _get_nc()
    res = run_bass_kernel_spmd(nc, _in_maps(q, W_qkv, b_qkv, W_proj),
                               core_ids=list(range(NCORES)))

    out = np.zeros((B, N, C), dtype=np.float32)
    for core in range(NCORES):
        out[core // HPC] += res.results[core]["y"]
    out += b_proj
    return out
